# revision 3
# baseline (speedup 1.0000x reference)
"""Trainium2 Bass kernel for CrossModalAttention (MHA + residual + LayerNorm).

Problem: B=4, L=2048, D=256, H=8, Dh=32.
In this environment the per-iteration cost is dominated by the axon
tunnel, not the NeuronCore: every dispatch costs a ~1.6ms RPC floor,
re-ships that core's input buffers, and multi-device dispatches
serialize (the 8-core data-parallel baseline paid 8 RPCs/iteration).
So this kernel minimizes dispatches and wire bytes:

  - ONE core runs all 4 batches in a single dispatch (N_CORES=1); the
    kernel itself is scalar-engine-bound (134M softmax exps at 128
    lanes @1.2GHz ~ 0.9ms), so losing data-parallelism costs far less
    than the extra RPCs would.
  - Everything on the wire is fp16: qT/kT/vT (channel-major), the four
    pre-transposed weights, and the output (upcast to fp32 on host).
    fp16 comfortably beats the 2e-2 gate (measured 1.0e-3); exp(s) <=
    exp(~10) ~ 2e4 < 65504 so no overflow without max-subtraction.
  - query is uploaded once (channel-major); the token-major copy needed
    for the residual is produced on-device by tensor-engine transposes
    (matmul against an uploaded [128,128] fp16 identity).
  - biases enter via tensor_scalar/tensor_tensor on the PSUM->SBUF
    copies instead of extra matmul passes; all matmuls are fp16
    (1 cycle/row vs 4 for fp32).

Per-batch dataflow (layouts as in the original 8-core baseline):
  QT = WqT.T @ qT  [256,2048] ch-major; KT likewise; V token-major into
  vaug[jt] = [V_h | ones] blocks; scoresT_h = KT_h.T @ QT_h packed 2
  heads/pass (tile_position rows), exp on ScalarE (PSUM->fp16 SBUF),
  PV accumulates [ctx;den] over 16 k-tiles, normalize = copy + recip +
  mult, out-proj + residual + LayerNorm -> fp16 out.
"""

import numpy as np

import concourse.bass as bass
import concourse.tile as tile
from concourse import bacc, mybir
from concourse.bass_utils import run_bass_kernel_spmd

F16 = mybir.dt.float16
F32 = mybir.dt.float32
D = 256
H = 8
DH = 32
L = 2048
P = 128
B = 4
SCALE = 1.0 / float(np.sqrt(DH))
LN_EPS = 1e-5

N_JT = L // P  # 16 k-token tiles
N_QC = L // 512  # 4 q chunks of 512
N_QT = L // P  # 16 q token tiles

N_CORES = 1  # single dispatch minimizes axon-tunnel RPC cost


def build_nc(bpc):
    nc = bacc.Bacc(None)

    qT_d = nc.declare_dram_parameter("qT", [bpc, D, L], F16, isOutput=False)
    kT_d = nc.declare_dram_parameter("kT", [bpc, D, L], F16, isOutput=False)
    vT_d = nc.declare_dram_parameter("vT", [bpc, D, L], F16, isOutput=False)
    wq_d = nc.declare_dram_parameter("WqT", [D, D], F16, isOutput=False)
    wk_d = nc.declare_dram_parameter("WkT", [D, D], F16, isOutput=False)
    wv_d = nc.declare_dram_parameter("WvT", [D, D], F16, isOutput=False)
    wo_d = nc.declare_dram_parameter("WoT", [D, D], F16, isOutput=False)
    # biascol[p, i, t] = bias_i[t*128+p] for i in (q, k); per-partition cols
    biascol_d = nc.declare_dram_parameter("biascol", [P, 2, 2], F32, isOutput=False)
    # rows broadcast across partitions for free-dim biases / LN params
    brow_d = nc.declare_dram_parameter("brow", [4, D], F32, isOutput=False)
    ident_d = nc.declare_dram_parameter("ident", [P, P], F16, isOutput=False)
    out_d = nc.declare_dram_parameter("out", [bpc, L, D], F16, isOutput=True)

    with tile.TileContext(nc) as tc:
        with (
            tc.tile_pool(name="singles", bufs=1) as singles,
            tc.tile_pool(name="inb", bufs=2) as inb,
            tc.tile_pool(name="work", bufs=1) as work,
            tc.tile_pool(name="temps", bufs=3) as temps,
            tc.tile_pool(name="mmps", bufs=2, space="PSUM") as mmps,
            tc.tile_pool(name="sps", bufs=2, space="PSUM") as sps,
            tc.tile_pool(name="pvps", bufs=1, space="PSUM") as pvps,
        ):
            # ---- constants / weights -------------------------------------
            wq_sb = singles.tile([P, 2, D], F16, tag="wq")
            wk_sb = singles.tile([P, 2, D], F16, tag="wk")
            wv_sb = singles.tile([P, 2, D], F16, tag="wv")
            wo_sb = singles.tile([P, 2, D], F16, tag="wo")
            for sb, d in ((wq_sb, wq_d), (wk_sb, wk_d), (wv_sb, wv_d), (wo_sb, wo_d)):
                nc.sync.dma_start(out=sb, in_=d.rearrange("(t p) j -> p t j", p=P))

            ident_sb = singles.tile([P, P], F16, tag="ident")
            nc.sync.dma_start(out=ident_sb, in_=ident_d[:, :])
            biascol_sb = singles.tile([P, 2, 2], F32, tag="biascol")
            nc.sync.dma_start(out=biascol_sb, in_=biascol_d[:, :, :])

            brow_sb = singles.tile([P, 4, D], F32, tag="brow")
            nc.gpsimd.dma_start(
                out=brow_sb, in_=brow_d[None, :, :].to_broadcast((P, 4, D))
            )
            bv_bc = brow_sb[:, 0, :]
            bo_bc = brow_sb[:, 1, :]
            lng_bc = brow_sb[:, 2, :]
            lnb_bc = brow_sb[:, 3, :]

            eps_sb = singles.tile([P, 1], F32, tag="eps")
            nc.vector.memset(eps_sb, LN_EPS)

            # ---- per-batch persistent working set ------------------------
            QT_sb = work.tile([P, 2, L], F16, tag="QT")
            KT_sb = work.tile([P, 2, L], F16, tag="KT")
            vaug = work.tile([P, N_JT, H * 64], F16, tag="vaug")
            ctxTn = work.tile([P, 2, L], F16, tag="ctxTn")
            y16 = work.tile([P, N_QT, D], F16, tag="y16")
            o16 = work.tile([P, N_QT, D], F16, tag="o16")
            mv_sb = work.tile([P, N_QT, 2], F32, tag="mv")
            sd_sb = work.tile([P, N_QT], F32, tag="sd")
            rstd_sb = work.tile([P, N_QT], F32, tag="rstd")

            for b in range(bpc):
                xq_sb = inb.tile([P, 2, L], F16, tag="xq")
                xk_sb = inb.tile([P, 2, L], F16, tag="xk")
                xv_sb = inb.tile([P, 2, L], F16, tag="xv")
                nc.sync.dma_start(
                    out=xq_sb, in_=qT_d[b].rearrange("(t p) l -> p t l", p=P)
                )
                nc.sync.dma_start(
                    out=xk_sb, in_=kT_d[b].rearrange("(t p) l -> p t l", p=P)
                )
                nc.sync.dma_start(
                    out=xv_sb, in_=vT_d[b].rearrange("(t p) l -> p t l", p=P)
                )

                # ---- QKV projections ------------------------------------
                for jt in range(2):
                    for qcc in range(4):
                        ps = mmps.tile([P, 512], F32, tag="mm")
                        nc.tensor.matmul(
                            ps,
                            lhsT=wq_sb[:, 0, jt * P : (jt + 1) * P],
                            rhs=xq_sb[:, 0, qcc * 512 : (qcc + 1) * 512],
                            start=True,
                            stop=False,
                        )
                        nc.tensor.matmul(
                            ps,
                            lhsT=wq_sb[:, 1, jt * P : (jt + 1) * P],
                            rhs=xq_sb[:, 1, qcc * 512 : (qcc + 1) * 512],
                            start=False,
                            stop=True,
                        )
                        nc.vector.tensor_scalar(
                            out=QT_sb[:, jt, qcc * 512 : (qcc + 1) * 512],
                            in0=ps,
                            scalar1=biascol_sb[:, 0, jt : jt + 1],
                            scalar2=None,
                            op0=mybir.AluOpType.add,
                        )
                for jt in range(2):
                    for kc in range(4):
                        ps = mmps.tile([P, 512], F32, tag="mm")
                        nc.tensor.matmul(
                            ps,
                            lhsT=wk_sb[:, 0, jt * P : (jt + 1) * P],
                            rhs=xk_sb[:, 0, kc * 512 : (kc + 1) * 512],
                            start=True,
                            stop=False,
                        )
                        nc.tensor.matmul(
                            ps,
                            lhsT=wk_sb[:, 1, jt * P : (jt + 1) * P],
                            rhs=xk_sb[:, 1, kc * 512 : (kc + 1) * 512],
                            start=False,
                            stop=True,
                        )
                        nc.vector.tensor_scalar(
                            out=KT_sb[:, jt, kc * 512 : (kc + 1) * 512],
                            in0=ps,
                            scalar1=biascol_sb[:, 1, jt : jt + 1],
                            scalar2=None,
                            op0=mybir.AluOpType.add,
                        )
                # V token-major, interleaved into vaug with ones blocks
                for tt in range(N_JT):
                    ps = mmps.tile([P, 512], F32, tag="mm")
                    nc.tensor.matmul(
                        ps[:, :D],
                        lhsT=xv_sb[:, 0, tt * P : (tt + 1) * P],
                        rhs=wv_sb[:, 0, :],
                        start=True,
                        stop=False,
                    )
                    nc.tensor.matmul(
                        ps[:, :D],
                        lhsT=xv_sb[:, 1, tt * P : (tt + 1) * P],
                        rhs=wv_sb[:, 1, :],
                        start=False,
                        stop=True,
                    )
                    vt = vaug[:, tt, :].rearrange("p (h c) -> p h c", c=64)
                    nc.vector.memset(vt[:, :, DH:], 1.0)
                    nc.vector.tensor_tensor(
                        out=vt[:, :, :DH],
                        in0=ps[:, :D].rearrange("p (h c) -> p h c", c=DH),
                        in1=bv_bc.rearrange("p (h c) -> p h c", c=DH),
                        op=mybir.AluOpType.add,
                    )

                # ---- attention ------------------------------------------
                for qc in range(N_QC):
                    q0 = qc * 512
                    for hp in range(4):  # head pairs (2hp, 2hp+1)
                        pv = pvps.tile([P, 2, 512], F32, tag="pv")
                        for jt in range(N_JT):
                            s = sps.tile([P, 2, 512], F32, tag="s")
                            for e in range(2):
                                h = 2 * hp + e
                                dt = h // 4
                                r0 = (h % 4) * DH
                                nc.tensor.matmul(
                                    s[:, e, :],
                                    lhsT=KT_sb[
                                        r0 : r0 + DH, dt, jt * P : (jt + 1) * P
                                    ],
                                    rhs=QT_sb[r0 : r0 + DH, dt, q0 : q0 + 512],
                                    start=True,
                                    stop=True,
                                    tile_position=(r0, 0),
                                )
                            es = temps.tile([P, 2, 512], F16, tag="es")
                            nc.scalar.activation(
                                out=es,
                                in_=s,
                                func=mybir.ActivationFunctionType.Exp,
                                scale=SCALE,
                            )
                            for e in range(2):
                                h = 2 * hp + e
                                nc.tensor.matmul(
                                    pv[0:64, e, :],
                                    lhsT=vaug[:, jt, 64 * h : 64 * h + 64],
                                    rhs=es[:, e, :],
                                    start=(jt == 0),
                                    stop=(jt == N_JT - 1),
                                )
                        for e in range(2):
                            h = 2 * hp + e
                            dt = h // 4
                            r0 = (h % 4) * DH
                            st = temps.tile([64, 512], F32, tag="st")
                            nc.vector.tensor_copy(out=st, in_=pv[0:64, e, :])
                            rec = temps.tile([DH, 512], F32, tag="rec")
                            nc.vector.reciprocal(out=rec, in_=st[DH:64, :])
                            nc.vector.tensor_tensor(
                                out=ctxTn[r0 : r0 + DH, dt, q0 : q0 + 512],
                                in0=st[0:DH, :],
                                in1=rec,
                                op=mybir.AluOpType.mult,
                            )

                # ---- residual transpose + out-proj + LN stats -----------
                for qt in range(N_QT):
                    # transpose via matmul: tp[:, ct*P:] = xq[:, ct, qtP].T @ I
                    tp = mmps.tile([P, 512], F32, tag="mm")
                    for ct in range(2):
                        nc.tensor.matmul(
                            tp[:, ct * P : (ct + 1) * P],
                            lhsT=xq_sb[:, ct, qt * P : (qt + 1) * P],
                            rhs=ident_sb,
                            start=True,
                            stop=True,
                        )
                    qres = temps.tile([P, D], F32, tag="qres")
                    nc.vector.tensor_tensor(
                        out=qres, in0=tp[:, :D], in1=bo_bc, op=mybir.AluOpType.add
                    )
                    po = mmps.tile([P, 512], F32, tag="mm")
                    nc.tensor.matmul(
                        po[:, :D],
                        lhsT=ctxTn[:, 0, qt * P : (qt + 1) * P],
                        rhs=wo_sb[:, 0, :],
                        start=True,
                        stop=False,
                    )
                    nc.tensor.matmul(
                        po[:, :D],
                        lhsT=ctxTn[:, 1, qt * P : (qt + 1) * P],
                        rhs=wo_sb[:, 1, :],
                        start=False,
                        stop=True,
                    )
                    nc.vector.tensor_tensor(
                        out=y16[:, qt, :],
                        in0=po[:, :D],
                        in1=qres,
                        op=mybir.AluOpType.add,
                    )
                    st6 = temps.tile([P, 6], F32, tag="st6")
                    nc.vector.bn_stats(out=st6, in_=y16[:, qt, :])
                    nc.vector.bn_aggr(out=mv_sb[:, qt, :], in_=st6)

                # ---- final LayerNorm pass -------------------------------
                nc.scalar.activation(
                    out=sd_sb,
                    in_=mv_sb[:, :, 1:2],
                    func=mybir.ActivationFunctionType.Sqrt,
                    bias=eps_sb,
                )
                nc.vector.reciprocal(out=rstd_sb, in_=sd_sb)
                for qt in range(N_QT):
                    yn = temps.tile([P, D], F32, tag="yn")
                    nc.vector.tensor_scalar(
                        out=yn,
                        in0=y16[:, qt, :],
                        scalar1=mv_sb[:, qt, 0:1],
                        scalar2=rstd_sb[:, qt : qt + 1],
                        op0=mybir.AluOpType.subtract,
                        op1=mybir.AluOpType.mult,
                    )
                    nc.vector.tensor_tensor(
                        out=yn, in0=yn, in1=lng_bc, op=mybir.AluOpType.mult
                    )
                    nc.vector.tensor_tensor(
                        out=o16[:, qt, :],
                        in0=yn,
                        in1=lnb_bc,
                        op=mybir.AluOpType.add,
                    )
                nc.sync.dma_start(
                    out=out_d[b].rearrange("(t p) d -> p t d", p=P), in_=o16
                )

    nc.finalize()
    return nc


_NC_CACHE = {}


def _get_nc(bpc=None):
    if bpc is None:
        bpc = B // N_CORES
    if bpc not in _NC_CACHE:
        _NC_CACHE[bpc] = build_nc(bpc)
    return _NC_CACHE[bpc]


def make_in_maps(query, key, value, Wq, bq, Wk, bk, Wv, bv, Wo, bo, ln_g, ln_b,
                 n_cores=None):
    if n_cores is None:
        n_cores = N_CORES
    bpc = B // n_cores
    f16 = lambda x: np.ascontiguousarray(np.asarray(x, dtype=np.float32)).astype(
        np.float16
    )
    f32 = lambda x: np.ascontiguousarray(np.asarray(x, dtype=np.float32))
    biascol = np.stack([np.asarray(bq), np.asarray(bk)]).reshape(2, 2, P)
    biascol = np.ascontiguousarray(biascol.transpose(2, 0, 1)).astype(np.float32)
    shared = {
        "WqT": f16(np.asarray(Wq).T),
        "WkT": f16(np.asarray(Wk).T),
        "WvT": f16(np.asarray(Wv).T),
        "WoT": f16(np.asarray(Wo).T),
        "biascol": biascol,
        "brow": f32(np.stack([np.asarray(bv), np.asarray(bo), np.asarray(ln_g), np.asarray(ln_b)])),
        "ident": np.eye(P, dtype=np.float16),
    }
    qT = f16(query).transpose(0, 2, 1)  # [B, D, L]
    kT = f16(key).transpose(0, 2, 1)
    vT = f16(value).transpose(0, 2, 1)
    in_maps = []
    for c in range(n_cores):
        bs = slice(c * bpc, (c + 1) * bpc)
        in_maps.append(
            dict(
                shared,
                qT=np.ascontiguousarray(qT[bs]),
                kT=np.ascontiguousarray(kT[bs]),
                vT=np.ascontiguousarray(vT[bs]),
            )
        )
    return in_maps


def kernel(query, key, value, Wq, bq, Wk, bk, Wv, bv, Wo, bo, ln_g, ln_b):
    bpc = B // N_CORES
    nc = _get_nc(bpc)
    in_maps = make_in_maps(
        query, key, value, Wq, bq, Wk, bk, Wv, bv, Wo, bo, ln_g, ln_b
    )
    res = run_bass_kernel_spmd(nc, in_maps, core_ids=list(range(N_CORES)))
    out = np.empty((B, L, D), dtype=np.float32)
    for c in range(N_CORES):
        out[c * bpc : (c + 1) * bpc] = np.asarray(
            res.results[c]["out"], dtype=np.float32
        )
    return out
